# revision 25
# baseline (speedup 1.0000x reference)
"""Causal self-attention Trainium2 Bass/Tile kernel.

Problem: B=4, T=2048, C=2048, H=16 heads, d=128. fp32 I/O.
Sharding over 8 cores: core i -> (batch b = i//2, head-group g = i%2).
Each core computes attention + partial c_proj for its 8 heads on its
batch; host sums the two head-group partials per batch.

Per-core math (HL = 8 local heads, ch = HL*d = 1024 local channels):
  V[t, ch]   = sum_c x[t, c] Wv[ch, c]          (A0)
  qT[dd, t]  = sum_c Wq[dd, c] x[t, c]          (A1, per head)
  kT[dd, t]  = sum_c Wk[dd, c] x[t, c]
  S[tq, tk]  = sum_dd qT[dd, tq] kT[dd, tk]     (B, causal row blocks)
  P          = exp(S / sqrt(d) + mask)           (no max-sub: logits bounded)
  attn[tq,d] = (sum_tk P[tq,tk] V[tk,d]) / l[tq]
  y[t, o]    = sum_ch attnT[ch, t] WpT[ch, o]   (C, partial over local ch)

All matmuls bf16 inputs, fp32 PSUM accumulation.

Scheduling: A1(h0) strips interleave with A0 blocks by x-quarter DMA
arrival; per head, S/exp (part A) runs one strip ahead of PV/L/normalize
(part B) so part B's exp-gated matmuls are always-ready PE filler; head
h's strips interleave head h+1's qk psum-groups. PE is ~96.5% busy over
the modeled span.
"""

import math
import sys

import numpy as np

sys.path.insert(0, "/opt/trn_rl_repo")

import ml_dtypes  # noqa: E402

import concourse.bass as bass  # noqa: E402
import concourse.bacc as bacc  # noqa: E402
import concourse.mybir as mybir  # noqa: E402
import concourse.tile as tile  # noqa: E402
from concourse.masks import make_identity  # noqa: E402

BF16 = mybir.dt.bfloat16
F32 = mybir.dt.float32
P = 128
SG = 512  # psum bank width in fp32


def _chunks(total, size):
    out = []
    o = 0
    while o < total:
        w = min(size, total - o)
        out.append((o, w))
        o += w
    return out


def build_nc(T=2048, C=2048, HL=8, d=128, reps=1, unroll=1, no_dma=False,
             phase="all", pss_bufs=2, psal_bufs=2):
    """Build the per-core Bass program (SPMD: same program on all cores).

    reps > 1 wraps the whole body in a device-side loop (timing only).
    unroll > 1 repeats the body at build time (model-only slope probe).
    no_dma skips every dma_start (diagnostic: pure compute/dependency time;
    results are garbage). phase: "all" | "noC" (skip c_proj) | "a01"
    (projections only, skip strips+c_proj) -- timing diagnostics.
    """
    TB = T // P  # token blocks
    CB = C // P  # contraction chunks over C
    CH = HL * d  # local channels
    scale = 1.0 / math.sqrt(d)
    QH = min(1024, T)  # qk psum tile width
    YH = min(1024, C)  # proj psum tile width

    nc = bacc.Bacc(None, target_bir_lowering=False)

    if no_dma:
        def dma(*a, **k):
            return None
    else:
        dma = nc.sync.dma_start

    xT = nc.dram_tensor("xT", [C, T], BF16, kind="ExternalInput")
    # wqk[p, cc, h, 0:d]=WqT chunk, [.., d:2d]=WkT chunk  (c = cc*128 + p)
    wqk = nc.dram_tensor("wqk", [P, CB, HL, 2 * d], BF16, kind="ExternalInput")
    wv = nc.dram_tensor("wv", [C, CH], BF16, kind="ExternalInput")
    wp = nc.dram_tensor("wp", [CH, C], BF16, kind="ExternalInput")
    y = nc.dram_tensor("y", [T, C], F32, kind="ExternalOutput")

    import contextlib

    with tile.TileContext(nc) as tc, contextlib.ExitStack() as es:
        if reps > 1:
            es.enter_context(tc.For_i(0, reps, 1))
        def _body():
            with (
              tc.tile_pool(name="const", bufs=1) as constp,
              tc.tile_pool(name="attn", bufs=HL) as attnp,
              tc.tile_pool(name="vsb", bufs=TB) as vpool,
          ):
              # ones column-block for row-sum matmuls: L = ones.T @ P
              ones = constp.tile([P, P], BF16)
              nc.gpsimd.memset(ones[:], 1.0)

              attnTs = []
              Vs = []

              with (
                  tc.tile_pool(name="xt", bufs=CB) as xtp,
                  tc.tile_pool(name="wqk", bufs=1) as wqkp,
                  tc.tile_pool(name="qk", bufs=4) as qkp,
                  tc.tile_pool(name="psqk", bufs=4, space="PSUM") as psqk,
              ):
                  QW = min(SG, T)  # tq strip width for PV/L accumulation

                  def emit_qk(h, qk_tiles, wqt=None, per_strip=False, granular=False):
                      """Eagerly issues the wqt DMA + tile allocs, returns a
                      generator yielding one A1 psum-group (or single matmul
                      when granular) per next() call."""
                      if wqt is None:
                          wqt = wqkp.tile([P, CB, 2 * d], BF16, tag="wqk")
                          dma(wqt[:], wqk[:, :, h, :])
                      qT = qkp.tile([P, T], BF16, tag="qT")
                      kT = qkp.tile([P, T], BF16, tag="kT")
                      qk_tiles[h] = (qT, kT)
                      return _emit_qk_gen(wqt, qT, kT, per_strip, granular)

                  def _emit_qk_gen(wqt, qT, kT, per_strip, granular):
                      if per_strip:
                          order = [
                              (mi, dst, ho, hw)
                              for ho, hw in _chunks(T, SG)
                              for mi, dst in ((0, qT), (1, kT))
                          ]
                      else:
                          # k first, then q: strip s of this head's pass
                          # consumes q cols [s*QW:(s+1)*QW] late, so q's tail
                          # groups can land last without stalling strip 0.
                          order = [
                              (mi, dst, ho, hw)
                              for mi, dst in ((1, kT), (0, qT))
                              for ho, hw in _chunks(T, SG)
                          ]
                      for mi, dst, ho, hw in order:
                          ps = psqk.tile([P, SG], F32, tag="qk")
                          for c in range(CB):
                              nc.tensor.matmul(
                                  ps[:, :hw],
                                  wqt[:, c, mi * d : (mi + 1) * d],
                                  xts[c][:, ho : ho + hw],
                                  start=(c == 0),
                                  stop=(c == CB - 1),
                                  skip_group_check=granular,
                              )
                              if granular:
                                  yield
                          nc.scalar.copy(dst[:, ho : ho + hw], ps[:, :hw])
                          if not granular:
                              yield

                  class StripA:
                      """Part A of one strip: S matmuls + mask + exp, emitted
                      block-by-block via emit_block so the scheduler can
                      interleave filler PE work between Act-paced blocks. DVE
                      folds the exp blocks into a running per-strip sum (lsum)
                      so part B needs a single ones-matmul for L."""

                      def __init__(self, h, q4, qk_tiles):
                          self.h = h
                          self.s0 = q4 * QW
                          self.s1 = self.s0 + QW
                          self.njs = self.s1 // P
                          self.qT, self.kT = qk_tiles[h]
                          self.pts = []
                          self.lsum = lsp.tile([P, QW], F32, tag="ls")
                          self.lsum16 = None

                      def emit_block(self, j):
                          s0, s1 = self.s0, self.s1
                          c0 = max(j * P, s0)
                          w = s1 - c0
                          ps = pss.tile([P, QW], F32, tag="st")
                          nc.tensor.matmul(
                              ps[:, :w],
                              self.kT[:, j * P : (j + 1) * P],
                              self.qT[:, c0:s1],
                              start=True,
                              stop=True,
                              skip_group_check=True,
                          )
                          pt = ptp.tile([P, QW], BF16, tag="pt")
                          nc.scalar.activation(
                              pt[:, :w],
                              ps[:, :w],
                              mybir.ActivationFunctionType.Exp,
                              scale=scale,
                          )
                          if j * P >= s0:
                              # diagonal block: zero future (tk > tq) entries
                              # of exp on the idle gpsimd engine, keeping DVE
                              # off the S->exp critical path
                              # keep where 1 - p + q > 0 (q >= p); walrus
                              # only implements is_gt here, not is_le
                              nc.gpsimd.affine_select(
                                  out=pt[:, 0:P],
                                  in_=pt[:, 0:P],
                                  compare_op=mybir.AluOpType.is_gt,
                                  fill=0.0,
                                  base=1,
                                  pattern=[[1, P]],
                                  channel_multiplier=-1,
                              )
                          off = c0 - s0
                          if j == 0:
                              nc.vector.tensor_copy(self.lsum[:, :], pt[:, :QW])
                          else:
                              nc.vector.tensor_add(
                                  self.lsum[:, off:QW],
                                  self.lsum[:, off:QW],
                                  pt[:, :w],
                              )
                          self.pts.append((pt, c0, w))

                      def finish(self):
                          self.lsum16 = ls16p.tile([P, QW], BF16, tag="ls16")
                          nc.scalar.copy(self.lsum16[:], self.lsum[:])

                  def emit_strip_b(st, attnT):
                      """Part B generator: PV matmuls (one yield each, used as
                      gap filler inside the next strip's part A), then the
                      single ones-matmul for L, reciprocal and normalize."""
                      h, s0, s1 = st.h, st.s0, st.s1
                      pa = psal.tile([P, QW], F32, tag="al")
                      for j, (pt, c0, w) in enumerate(st.pts):
                          nc.tensor.matmul(
                              pa[:, c0 - s0 : s1 - s0],
                              Vs[j][:, h * d : (h + 1) * d],
                              pt[:, :w],
                              start=(j == 0),
                              stop=(j == st.njs - 1),
                              skip_group_check=True,
                          )
                          yield
                      pl = psal.tile([P, QW], F32, tag="al")
                      nc.tensor.matmul(
                          pl[:],
                          ones[:],
                          st.lsum16[:],
                          start=True,
                          stop=True,
                          skip_group_check=True,
                      )
                      yield
                      rl = rlp.tile([P, QW], F32, tag="rl")
                      nc.vector.reciprocal(rl[:], pl[:])
                      nc.vector.tensor_tensor(
                          out=attnT[:, s0:s1],
                          in0=pa[:],
                          in1=rl[:],
                          op=mybir.AluOpType.mult,
                      )

                  # DMA order: wqt(h0), xt quarter 0, wv, xt quarters 1-3.
                  # A1(h0) strip s and A0 blocks m=4s..4s+3 only need x
                  # quarter s, so interleaving them keeps PE fed while wv
                  # and the later x quarters stream in underneath.
                  xts = [
                      xtp.tile([P, T], BF16, name=f"xt{c}", tag="xt")
                      for c in range(CB)
                  ]
                  wqt0 = wqkp.tile([P, CB, 2 * d], BF16, tag="wqk")
                  dma(wqt0[:], wqk[:, :, 0, :])
                  QTR = max(T // 4, P)
                  for c in range(CB):
                      dma(
                          xts[c][:, 0:QTR], xT[c * P : (c + 1) * P, 0:QTR]
                      )
                  with (
                      tc.tile_pool(name="wv", bufs=CB) as wvp,
                      tc.tile_pool(name="psa0", bufs=4, space="PSUM") as psa0p,
                  ):
                      wvts = []
                      for c in range(CB):
                          wvt = wvp.tile([P, CH], BF16)
                          dma(wvt[:], wv[c * P : (c + 1) * P, :])
                          wvts.append(wvt)
                      for qo in range(QTR, T, QTR):
                          for c in range(CB):
                              dma(
                                  xts[c][:, qo : qo + QTR],
                                  xT[c * P : (c + 1) * P, qo : qo + QTR],
                              )

                      def emit_a0():
                          # Phase A0: V = x @ Wv.T, per-(m,o) accumulation
                          # groups through a 4-deep PSUM pool (deeper rotation
                          # absorbs the drain-copy latency between groups)
                          for m in range(TB):
                              for o, w in _chunks(CH, SG):
                                  ps = psa0p.tile(
                                      [P, SG], F32, tag="a0", name=f"a0_{m}_{o}"
                                  )
                                  for c in range(CB):
                                      nc.tensor.matmul(
                                          ps[:, :w],
                                          xts[c][:, m * P : (m + 1) * P],
                                          wvts[c][:, o : o + w],
                                          start=(c == 0),
                                          stop=(c == CB - 1),
                                      )
                                  if o == 0:
                                      V = vpool.tile([P, CH], BF16, tag="V")
                                      Vs.append(V)
                                  nc.scalar.copy(Vs[m][:, o : o + w], ps[:, :w])
                                  yield

                      # interleave A1(h0) with A0 by x-quarter availability
                      qk_tiles = {}
                      qk0 = emit_qk(0, qk_tiles, wqt0, per_strip=True)
                      a0 = emit_a0()
                      nqtr = T // QTR
                      for s in range(nqtr):
                          next(qk0, None)
                          next(qk0, None)
                          for _ in range(2 * (TB // nqtr)):
                              next(a0, None)
                      for _ in qk0:
                          pass
                      for _ in a0:
                          pass

                  if phase == "a01":
                      for h in range(1, HL):
                          g = emit_qk(h, qk_tiles)
                          for _ in g:
                              pass
                  # software pipeline: part B (PV/L) lags part A (S/exp) by
                  # one strip. With pss runway exhausted, S block j>=runway
                  # stalls PE on the Act-paced exp drain of block j-runway, so
                  # the scheduler interleaves single always-ready matmuls
                  # (prev-strip PV, next-head qk chunks) between S blocks to
                  # fill those gaps; remaining qk work drains between strips
                  # so qT/kT(h+1) completes before head h ends.
                  n_strips = T // QW if phase != "a01" else 0
                  with (
                      tc.tile_pool(name="pt", bufs=TB + 6) as ptp,
                      tc.tile_pool(name="rl", bufs=2) as rlp,
                      tc.tile_pool(name="ls", bufs=2) as lsp,
                      tc.tile_pool(name="ls16", bufs=3) as ls16p,
                      tc.tile_pool(
                          name="pss", bufs=pss_bufs, space="PSUM"
                      ) as pss,
                      tc.tile_pool(
                          name="psal", bufs=psal_bufs, space="PSUM"
                      ) as psal,
                  ):
                      # after strip q4, total granular qk pulls must reach
                      # quota[q4] so all 8 groups (128 matmuls) finish by the
                      # end of strip 2, matching the baseline's coarse sched.
                      quota = [48, 96, 128, 128]
                      bq = None  # part-B generator of the previous strip
                      attnT = None
                      for h in range(HL if n_strips else 0):
                          nxt = (
                              emit_qk(h + 1, qk_tiles, granular=True)
                              if h + 1 < HL
                              else iter(())
                          )
                          pulled = 0
                          attnT = attnp.tile([P, T], BF16, tag="attnT")
                          for q4 in range(n_strips):
                              st = StripA(h, q4, qk_tiles)
                              for j in range(st.njs):
                                  st.emit_block(j)
                                  if j >= 2:
                                      # ~2 fillers cover the Act-vs-PE pace
                                      # gap per throttled block
                                      for _ in range(2):
                                          if bq is not None and next(bq, -1) != -1:
                                              continue
                                          bq = None
                                          if next(nxt, -1) != -1:
                                              pulled += 1
                              st.finish()
                              if bq is not None:
                                  for _ in bq:
                                      pass
                              bq = emit_strip_b(st, attnT)
                              while pulled < quota[q4] and next(nxt, -1) != -1:
                                  pulled += 1
                          for _ in nxt:
                              pass
                          attnTs.append(attnT)
                          qk_tiles.pop(h, None)
                      if bq is not None:
                          for _ in bq:
                              pass

              # ---------------- Phase C: y = attn @ Wp.T (partial) --------
              if phase != "all":
                  return
              with (
                  tc.tile_pool(name="wp", bufs=CH // P) as wpp,
                  tc.tile_pool(name="ysb", bufs=2) as ysbp,
                  tc.tile_pool(name="psy", bufs=2, space="PSUM") as psy,
              ):
                  wpts = []
                  for ch in range(CH // P):
                      wpt = wpp.tile([P, C], BF16, tag="wp")
                      dma(wpt[:], wp[ch * P : (ch + 1) * P, :])
                      wpts.append(wpt)
                  for tb in range(TB):
                      ysb = ysbp.tile([P, C], F32, tag="ysb")
                      for ho, hw in _chunks(C, YH):
                          ps = psy.tile([P, YH], F32, tag="y")
                          for ch in range(CH // P):
                              hd = ch * P // d  # owning local head of this chunk
                              for o, w in _chunks(hw, SG):
                                  nc.tensor.matmul(
                                      ps[:, o : o + w],
                                      attnTs[hd][:, tb * P : (tb + 1) * P],
                                      wpts[ch][:, ho + o : ho + o + w],
                                      start=(ch == 0),
                                      stop=(ch == CH // P - 1),
                                  )
                          nc.vector.tensor_copy(ysb[:, ho : ho + hw], ps[:, :hw])
                      dma(y[tb * P : (tb + 1) * P, :], ysb[:])

        for _u in range(unroll):
            _body()

    return nc


def make_core_inputs(x, W_attn, W_proj, b, g, T=2048, C=2048, HL=8, d=128):
    """Host-side shard + layout prep for core (batch b, head-group g)."""
    bf16 = ml_dtypes.bfloat16
    CB = C // P
    CH = HL * d
    xb = np.asarray(x[b], dtype=np.float32)  # [T, C]
    xT = np.ascontiguousarray(xb.T).astype(bf16)  # [C, T]

    q_rows = W_attn[g * CH : (g + 1) * CH, :]  # [CH, C]
    k_rows = W_attn[C + g * CH : C + (g + 1) * CH, :]
    v_rows = W_attn[2 * C + g * CH : 2 * C + (g + 1) * CH, :]

    # wqk[p, cc, h, col]: col 0:d -> WqT, d:2d -> WkT; c = cc*128 + p
    wqkT = np.empty((C, HL, 2 * d), dtype=np.float32)
    for h in range(HL):
        wqkT[:, h, :d] = q_rows[h * d : (h + 1) * d, :].T
        wqkT[:, h, d:] = k_rows[h * d : (h + 1) * d, :].T
    wqk = np.ascontiguousarray(
        wqkT.reshape(CB, P, HL, 2 * d).transpose(1, 0, 2, 3)
    ).astype(bf16)

    wv = np.ascontiguousarray(v_rows.T).astype(bf16)  # [C, CH]
    wpm = np.ascontiguousarray(W_proj[:, g * CH : (g + 1) * CH].T).astype(
        bf16
    )  # [CH, C]
    return {"xT": xT, "wqk": wqk, "wv": wv, "wp": wpm}


def core_reference(xT, wqk, wv, wp, T=2048, C=2048, HL=8, d=128):
    """Numpy replica of one core's program (bf16 inputs, fp32 accum)."""
    CB = C // P
    xTf = np.asarray(xT, dtype=np.float32)
    x = xTf.T  # [T, C]
    wqkf = np.asarray(wqk, dtype=np.float32).transpose(1, 0, 2, 3).reshape(C, HL, 2 * d)
    V = x @ np.asarray(wv, dtype=np.float32)  # [T, CH]
    out = np.zeros((T, HL * d), dtype=np.float32)
    for h in range(HL):
        q = x @ wqkf[:, h, :d]  # [T, d]
        k = x @ wqkf[:, h, d:]
        S = (q @ k.T) / math.sqrt(d)
        mask = np.triu(np.ones((T, T), dtype=bool), 1)
        S = np.where(mask, -np.inf, S)
        Pm = np.exp(S)
        Pm = Pm / Pm.sum(-1, keepdims=True)
        out[:, h * d : (h + 1) * d] = Pm @ V[:, h * d : (h + 1) * d]
    return out @ np.asarray(wp, dtype=np.float32)  # [T, C] partial


_CACHE = {}


def _get_nc():
    if "nc" not in _CACHE:
        nc = build_nc()
        nc.compile()
        _CACHE["nc"] = nc
    return _CACHE["nc"]


def run_cores(in_maps, trace=False):
    from concourse.bass_utils import run_bass_kernel_spmd

    nc = _get_nc()
    return run_bass_kernel_spmd(nc, in_maps, list(range(len(in_maps))), trace=trace)


def kernel(x, W_attn, W_proj):
    x = np.asarray(x, dtype=np.float32)
    W_attn = np.asarray(W_attn, dtype=np.float32)
    W_proj = np.asarray(W_proj, dtype=np.float32)
    B = x.shape[0]
    in_maps = [make_core_inputs(x, W_attn, W_proj, i // 2, i % 2) for i in range(8)]
    res = run_cores(in_maps).results
    y = np.stack(
        [res[2 * b]["y"].astype(np.float32) + res[2 * b + 1]["y"] for b in range(B)]
    )
    return y

